# revision 10
# baseline (speedup 1.0000x reference)
"""Sigmoid-attention MHA kernel for 8 Trainium2 NeuronCores (v2).

Problem: x[4,2048,512], W_q/W_k/W_v/W_o[512,512] (already scaled).
  Q = x@Wq.T, K = x@Wk.T, V = x@Wv.T split into 8 heads of depth 64
  attn = sigmoid(QK^T/sqrt(64) - log(2048));  out = (attn@V merged)@Wo.T

Sharding: core c handles batch b=c//2, head-group g=c%2 (4 heads = 2 pairs).
Each core computes a partial output projection over its 256 head-features;
host sums the two partials per batch.

v2 design (ScalarE sigmoid is the bottleneck: ~16.8M activations/core at
1 elem/cycle/lane @1.2GHz ~= 110us + per-instruction overhead):
  - PSUM (8 banks): score ring rA[128,1536](3) + rB[128,1024](2), one
    projection bank pj[128,512], two col-tiled attnV banks oA/oB.
  - Bigger ACTIVATE blocks (1536/1024 alternating) cut per-call overhead.
  - Scores row-tiled (two 64-contraction heads in PE row halves), attnV
    col-tiled (head outputs at PSUM partitions 0-63 / 64-127) -> PE ~2x
    on attention matmuls, keeping PE well under ScalarE.
  - x DMA'd in query-chunk order; only K/Q(pair0,qc0) block the first
    scores; all other projections stream on the pj bank concurrently
    with attention.
  - Output projection runs as [128,512] half-waves on the pj bank as soon
    as each pair-1 query chunk retires; the last 4 rotate over free banks.
  - attn/V in bf16 (fp32 weight load fails the ISA check under column
    tiling; fp32 PSUM accumulation keeps rounding benign).
"""

import os
import numpy as np

DEBUG = bool(int(os.environ.get("KERNEL_DEBUG", "0")))
LOOP = int(os.environ.get("KERNEL_LOOP", "0"))  # >0: wrap body in For_i (timing)
ABUFS = int(os.environ.get("KERNEL_ABUFS", "6"))

B, S, D = 4, 2048, 512
NH, DEPTH = 8, 64
G = 2          # head groups (one per core pair)
GF = 256       # features per group
NEG_LOG_S = float(np.float32(-np.log(np.float32(S))))
INV_SQRT_DK = 0.125

_CACHE = {}


def _build_nc():
    import concourse.bacc as bacc
    import concourse.tile as tile
    from concourse import mybir

    f32 = mybir.dt.float32
    f32r = mybir.dt.float32r
    bf16 = mybir.dt.bfloat16
    nc = bacc.Bacc("TRN2", target_bir_lowering=False, debug=False, num_devices=8)

    xt_d = nc.dram_tensor("xt", [128, 8192], f32r, kind="ExternalInput").ap()
    wq_d = nc.dram_tensor("wq", [128, 1024], f32r, kind="ExternalInput").ap()
    wk_d = nc.dram_tensor("wk", [128, 1024], f32r, kind="ExternalInput").ap()
    wv_d = nc.dram_tensor("wv", [128, 1024], f32r, kind="ExternalInput").ap()
    wo_d = nc.dram_tensor("wo", [128, 1024], f32r, kind="ExternalInput").ap()
    out_d = nc.dram_tensor("out", [S, D], f32, kind="ExternalOutput").ap()
    dbg = {}
    if DEBUG:
        for nm in ("qt", "kt"):
            dbg[nm] = [nc.dram_tensor(f"dbg_{nm}{m}", [128, 2048], f32r,
                                      kind="ExternalOutput").ap() for m in range(2)]
        dbg["ot"] = [nc.dram_tensor(f"dbg_ot{m}", [128, 2048], f32r,
                                    kind="ExternalOutput").ap() for m in range(2)]
        dbg["v"] = [nc.dram_tensor(f"dbg_v{j}", [128, 512], bf16,
                                   kind="ExternalOutput").ap() for j in range(8)]

    with tile.TileContext(nc) as tc:
        with (
            tc.tile_pool(name="persist", bufs=1) as persist,
            tc.tile_pool(name="attn", bufs=ABUFS) as apool,
            tc.tile_pool(name="stage", bufs=3) as stage,
            tc.tile_pool(name="rpsum", bufs=1, space="PSUM") as rpsum,
            tc.tile_pool(name="ppsum", bufs=1, space="PSUM") as ppsum,
            tc.tile_pool(name="opsum", bufs=1, space="PSUM") as opsum,
        ):
            import contextlib
            if LOOP > 0:
                loop_cm = tc.For_i(0, LOOP, 1)
            else:
                loop_cm = contextlib.nullcontext()
            Sig = mybir.ActivationFunctionType.Sigmoid

            def mm(out, lhsT, rhs, start, stop):
                # float32r: single-pass fp32 matmul, slightly reduced
                # multiply precision
                nc.tensor.matmul(out, lhsT=lhsT.bitcast(f32r),
                                 rhs=rhs.bitcast(f32r), start=start, stop=stop)

            def mmb(out, lhsT, rhs, start, stop):
                nc.tensor.matmul(out, lhsT=lhsT, rhs=rhs,
                                 start=start, stop=stop)

            with loop_cm:
                bias_t = persist.tile([128, 1], f32, tag="bias", name="bias_t")
                nc.vector.memset(bias_t[:], NEG_LOG_S)
                warm_t = persist.tile([128, 1], f32, tag="warm", name="warm_t")
                nc.scalar.activation(warm_t[:], bias_t[:], Sig, bias=bias_t[:])

                wq_sb = persist.tile([128, 1024], f32r, tag="wq", name="wq_sb")
                wk_sb = persist.tile([128, 1024], f32r, tag="wk", name="wk_sb")
                wv_sb = persist.tile([128, 1024], f32r, tag="wv", name="wv_sb")
                wo_sb = persist.tile([128, 1024], f32r, tag="wo", name="wo_sb")
                xt = [persist.tile([128, 2048], f32r, tag=f"xt{c}", name=f"xt{c}")
                      for c in range(4)]
                # Weights for K/Q first, then x in query-chunk-major order so
                # the pair-0 qc0 projections can start after ~1MiB of x.
                nc.sync.dma_start(out=wk_sb[:], in_=wk_d[:])
                nc.sync.dma_start(out=wq_sb[:], in_=wq_d[:])
                for qc in range(4):
                    for c in range(4):
                        cs = slice(512 * qc, 512 * (qc + 1))
                        nc.sync.dma_start(out=xt[c][:, cs],
                                          in_=xt_d[:, 2048 * c + 512 * qc:
                                                   2048 * c + 512 * (qc + 1)])
                nc.sync.dma_start(out=wv_sb[:], in_=wv_d[:])
                nc.sync.dma_start(out=wo_sb[:], in_=wo_d[:])

                qt = [persist.tile([128, 2048], f32r, tag=f"qt{m}", name=f"qt{m}")
                      for m in range(2)]
                kt = [persist.tile([128, 2048], f32r, tag=f"kt{m}", name=f"kt{m}")
                      for m in range(2)]
                v2 = [persist.tile([128, 512], bf16, tag=f"v{j}", name=f"v{j}")
                      for j in range(8)]
                ot = [persist.tile([128, 2048], f32r, tag=f"ot{m}", name=f"ot{m}")
                      for m in range(2)]

                # ---- projection tasks ----
                def proj_kq(w_sb, dst, p, qc, pool, tag):
                    ps = pool.tile([128, 512], f32, tag=tag, name="pp")
                    for vkc in range(4):
                        mm(ps[:, 0:512],
                           w_sb[:, 256 * vkc + 128 * p:256 * vkc + 128 * p + 128],
                           xt[vkc][:, 512 * qc:512 * (qc + 1)],
                           start=(vkc == 0), stop=(vkc == 3))
                    nc.vector.tensor_copy(dst[:, 512 * qc:512 * (qc + 1)],
                                          ps[:, 0:512])

                def proj_v(j, pool, tag):
                    # V for key chunks (2j, 2j+1): two sequential accumulation
                    # groups in one bank (second start=True only resets
                    # has_written bits; finished values in cols 0:256 persist)
                    ps = pool.tile([128, 512], f32, tag=tag, name="pv")
                    for half in range(2):
                        kc = 2 * j + half
                        for vkc in range(4):
                            mm(ps[:, 256 * half:256 * half + 256],
                               xt[vkc][:, 128 * kc:128 * (kc + 1)],
                               wv_sb[:, 256 * vkc:256 * (vkc + 1)],
                               start=(vkc == 0), stop=(vkc == 3))
                    nc.vector.tensor_copy(v2[j][:], ps[:, 0:512])

                # Prologue: the only projections the first scores block on —
                # rotate over three free banks for overlap.
                proj_kq(wk_sb, kt[0], 0, 0, ppsum, "pj")
                proj_kq(wq_sb, qt[0], 0, 0, rpsum, "rA")
                proj_kq(wk_sb, kt[0], 0, 1, rpsum, "rB")
                # Background: stream on the pj bank, paced by its own
                # mm->evac chain, concurrent with attention.
                bg = [("k", 0, 2), ("k", 0, 3),
                      ("v", 0), ("v", 1), ("v", 2), ("q", 0, 1),
                      ("v", 3), ("v", 4), ("v", 5), ("v", 6), ("v", 7),
                      ("q", 0, 2), ("q", 0, 3),
                      ("k", 1, 0), ("k", 1, 1), ("k", 1, 2), ("k", 1, 3),
                      ("q", 1, 0), ("q", 1, 1), ("q", 1, 2), ("q", 1, 3)]
                for t in bg:
                    if t[0] == "k":
                        proj_kq(wk_sb, kt[t[1]], t[1], t[2], ppsum, "pj")
                    elif t[0] == "q":
                        proj_kq(wq_sb, qt[t[1]], t[1], t[2], ppsum, "pj")
                    else:
                        proj_v(t[1], ppsum, "pj")

                # ---- output projection half-waves ([128 tokens, 512]) ----
                TAILROT = [(ppsum, "pj"), (rpsum, "rA"), (rpsum, "rB"),
                           (ppsum, "pj")]

                def p_half(t2, last=False):
                    pool, tag = TAILROT[t2 % 4] if last else (ppsum, "pj")
                    ps = pool.tile([128, 512], f32, tag=tag, name="pw")
                    for c in range(2):
                        mm(ps[:, 0:512], ot[c][:, 128 * t2:128 * (t2 + 1)],
                           wo_sb[:, 512 * c:512 * (c + 1)],
                           start=(c == 0), stop=(c == 1))
                    st = stage.tile([128, 512], f32, tag="pstage", name="pstage")
                    nc.vector.tensor_copy(st[:], ps[:, 0:512])
                    nc.sync.dma_start(out=out_d[128 * t2:128 * (t2 + 1), :],
                                      in_=st[:])

                # ---- attention ----
                SIZES = [3, 2] * 6 + [2]  # units of 512 cols; 3->rA, 2->rB

                def attention(p):
                    for qc in range(4):
                        qs = slice(512 * qc, 512 * (qc + 1))
                        psA = opsum.tile([128, 512], f32, tag="oA", name="psA")
                        psB = opsum.tile([128, 512], f32, tag="oB", name="psB")
                        units = [(kc, h) for kc in range(16) for h in (0, 1)]
                        ui = 0
                        for bs in SIZES:
                            tag = "rA" if bs == 3 else "rB"
                            atag = "aA" if bs == 3 else "aB"
                            cur = units[ui:ui + bs]
                            ui += bs
                            s = rpsum.tile([128, 512 * bs], f32, tag=tag,
                                           name="ps")
                            for ci, (kc, h) in enumerate(cur):
                                mm(s[:, 512 * ci:512 * (ci + 1)],
                                   kt[p][64 * h:64 * h + 64,
                                         128 * kc:128 * (kc + 1)],
                                   qt[p][64 * h:64 * h + 64, qs],
                                   start=True, stop=True)
                            a = apool.tile([128, 512 * bs], bf16, tag=atag,
                                           name="attn")
                            nc.scalar.activation(a[:], s[:], Sig,
                                                 bias=bias_t[:],
                                                 scale=INV_SQRT_DK)
                            for ci, (kc, h) in enumerate(cur):
                                psX = psA if h == 0 else psB
                                vcol = 256 * (kc % 2) + 128 * p + 64 * h
                                mmb(psX[64 * h:64 * h + 64, 0:512],
                                    v2[kc // 2][:, vcol:vcol + 64],
                                    a[:, 512 * ci:512 * (ci + 1)],
                                    start=(kc == 0), stop=(kc == 15))
                        nc.vector.tensor_copy(ot[p][0:64, qs], psA[0:64, 0:512])
                        nc.vector.tensor_copy(ot[p][64:128, qs],
                                              psB[64:128, 0:512])
                        if p == 1:
                            for t2 in range(4 * qc, 4 * qc + 4):
                                p_half(t2, last=(qc == 3))

                attention(0)
                attention(1)

                if DEBUG:
                    for m in range(2):
                        nc.sync.dma_start(out=dbg["qt"][m], in_=qt[m][:])
                        nc.sync.dma_start(out=dbg["kt"][m], in_=kt[m][:])
                        nc.sync.dma_start(out=dbg["ot"][m], in_=ot[m][:])
                    for j in range(8):
                        nc.sync.dma_start(out=dbg["v"][j], in_=v2[j][:])

    nc.compile()
    return nc


def get_nc():
    if "nc" not in _CACHE:
        _CACHE["nc"] = _build_nc()
    return _CACHE["nc"]


def make_in_maps(x, W_q, W_k, W_v, W_o):
    x = np.ascontiguousarray(np.asarray(x, dtype=np.float32))
    ws = [np.asarray(w, dtype=np.float32) for w in (W_q, W_k, W_v, W_o)]
    W_q, W_k, W_v, W_o = ws

    def chunked(a, nchunks):
        # [128*nchunks, m] -> [128, nchunks*m] with chunk-major columns
        m = a.shape[1]
        return np.ascontiguousarray(
            a.reshape(nchunks, 128, m).transpose(1, 0, 2).reshape(128, nchunks * m))

    in_maps = []
    for c in range(8):
        b, g = divmod(c, 2)
        gf = slice(GF * g, GF * (g + 1))
        in_maps.append({
            "xt": chunked(np.ascontiguousarray(x[b].T), 4),
            "wq": chunked(np.ascontiguousarray(W_q[gf, :].T), 4),
            "wk": chunked(np.ascontiguousarray(W_k[gf, :].T), 4),
            "wv": chunked(np.ascontiguousarray(W_v[gf, :].T), 4),
            "wo": chunked(np.ascontiguousarray(W_o[:, gf].T), 2),
        })
    return in_maps


def kernel(x, W_q, W_k, W_v, W_o):
    from concourse.bass_utils import run_bass_kernel_spmd

    nc = get_nc()
    in_maps = make_in_maps(x, W_q, W_k, W_v, W_o)
    res = run_bass_kernel_spmd(nc, in_maps, list(range(8)))
    parts = [res.results[c]["out"] for c in range(8)]
    out = np.stack([parts[2 * b] + parts[2 * b + 1] for b in range(B)])
    return np.ascontiguousarray(out.astype(np.float32))


# revision 11
# speedup vs baseline: 1.3087x; 1.3087x over previous
"""Sigmoid-attention MHA kernel for 8 Trainium2 NeuronCores (v6).

Problem: x[4,2048,512], W_q/W_k/W_v/W_o[512,512] (already scaled).
  Q = x@Wq.T, K = x@Wk.T, V = x@Wv.T split into 8 heads of depth 64
  attn = sigmoid(QK^T/sqrt(64) - log(2048));  out = (attn@V merged)@Wo.T

Sharding: core c handles batch b=c//2, head-group g=c%2 (4 heads = 2 pairs).
Each core computes a partial output projection over its 256 head-features;
host sums the two partials per batch.

v2 design (ScalarE sigmoid is the bottleneck: ~16.8M activations/core at
1 elem/cycle/lane @1.2GHz ~= 110us + per-instruction overhead):
  - PSUM (8 banks): score ring rA[128,1536](3) + rB[128,1024](2), one
    projection bank pj[128,512], two col-tiled attnV banks oA/oB.
  - Bigger ACTIVATE blocks (1536/1024 alternating) cut per-call overhead.
  - Scores row-tiled (two 64-contraction heads in PE row halves), attnV
    col-tiled (head outputs at PSUM partitions 0-63 / 64-127) -> PE ~2x
    on attention matmuls, keeping PE well under ScalarE.
  - x DMA'd in query-chunk order; only K/Q(pair0,qc0) block the first
    scores; all other projections stream on the pj bank concurrently
    with attention.
  - Output projection runs as [128,512] half-waves on the pj bank as soon
    as each pair-1 query chunk retires; the last 4 rotate over free banks.
  - attn/V in bf16 (fp32 weight load fails the ISA check under column
    tiling; fp32 PSUM accumulation keeps rounding benign).
"""

import os
import numpy as np

DEBUG = bool(int(os.environ.get("KERNEL_DEBUG", "0")))
LOOP = int(os.environ.get("KERNEL_LOOP", "0"))  # >0: wrap body in For_i (timing)
ABUFS = int(os.environ.get("KERNEL_ABUFS", "6"))

B, S, D = 4, 2048, 512
NH, DEPTH = 8, 64
G = 2          # head groups (one per core pair)
GF = 256       # features per group
NEG_LOG_S = float(np.float32(-np.log(np.float32(S))))
INV_SQRT_DK = 0.125

_CACHE = {}


def _build_nc():
    import concourse.bacc as bacc
    import concourse.tile as tile
    from concourse import mybir

    f32 = mybir.dt.float32
    f32r = mybir.dt.float32r
    bf16 = mybir.dt.bfloat16
    nc = bacc.Bacc("TRN2", target_bir_lowering=False, debug=False, num_devices=8)

    xt_d = nc.dram_tensor("xt", [128, 8192], f32r, kind="ExternalInput").ap()
    wq_d = nc.dram_tensor("wq", [128, 1024], f32r, kind="ExternalInput").ap()
    wk_d = nc.dram_tensor("wk", [128, 1024], f32r, kind="ExternalInput").ap()
    wv_d = nc.dram_tensor("wv", [128, 1024], f32r, kind="ExternalInput").ap()
    wo_d = nc.dram_tensor("wo", [128, 1024], f32r, kind="ExternalInput").ap()
    out_d = nc.dram_tensor("out", [S, D], f32, kind="ExternalOutput").ap()
    dbg = {}
    if DEBUG:
        for nm in ("qt", "kt"):
            dbg[nm] = [nc.dram_tensor(f"dbg_{nm}{m}", [128, 2048], f32r,
                                      kind="ExternalOutput").ap() for m in range(2)]
        dbg["ot"] = [nc.dram_tensor(f"dbg_ot{m}", [128, 2048], f32r,
                                    kind="ExternalOutput").ap() for m in range(2)]
        dbg["v"] = [nc.dram_tensor(f"dbg_v{j}", [128, 512], bf16,
                                   kind="ExternalOutput").ap() for j in range(8)]

    with tile.TileContext(nc) as tc:
        with (
            tc.tile_pool(name="persist", bufs=1) as persist,
            tc.tile_pool(name="attn", bufs=ABUFS) as apool,
            tc.tile_pool(name="stage", bufs=3) as stage,
            tc.tile_pool(name="rpsum", bufs=1, space="PSUM") as rpsum,
            tc.tile_pool(name="ppsum", bufs=1, space="PSUM") as ppsum,
            tc.tile_pool(name="opsum", bufs=1, space="PSUM") as opsum,
        ):
            import contextlib
            if LOOP > 0:
                loop_cm = tc.For_i(0, LOOP, 1)
            else:
                loop_cm = contextlib.nullcontext()
            Sig = mybir.ActivationFunctionType.Sigmoid

            def mm(out, lhsT, rhs, start, stop):
                # float32r: single-pass fp32 matmul, slightly reduced
                # multiply precision
                nc.tensor.matmul(out, lhsT=lhsT.bitcast(f32r),
                                 rhs=rhs.bitcast(f32r), start=start, stop=stop)

            def mmb(out, lhsT, rhs, start, stop):
                nc.tensor.matmul(out, lhsT=lhsT, rhs=rhs,
                                 start=start, stop=stop)

            with loop_cm:
                bias_t = persist.tile([128, 1], f32, tag="bias", name="bias_t")
                nc.vector.memset(bias_t[:], NEG_LOG_S)
                warm_t = persist.tile([128, 1], f32, tag="warm", name="warm_t")
                nc.scalar.activation(warm_t[:], bias_t[:], Sig, bias=bias_t[:])

                wq_sb = persist.tile([128, 1024], f32r, tag="wq", name="wq_sb")
                wk_sb = persist.tile([128, 1024], f32r, tag="wk", name="wk_sb")
                wv_sb = persist.tile([128, 1024], f32r, tag="wv", name="wv_sb")
                wo_sb = persist.tile([128, 1024], f32r, tag="wo", name="wo_sb")
                xt = [persist.tile([128, 2048], f32r, tag=f"xt{c}", name=f"xt{c}")
                      for c in range(4)]
                # Weights for K/Q first, then x in query-chunk-major order so
                # the pair-0 qc0 projections can start after ~1MiB of x.
                # Weights stream on the SP queue while x streams on the
                # (otherwise idle) Pool queue, query-chunk-major.
                nc.sync.dma_start(out=wk_sb[:], in_=wk_d[:])
                nc.sync.dma_start(out=wq_sb[:], in_=wq_d[:])
                nc.sync.dma_start(out=wv_sb[:], in_=wv_d[:])
                nc.sync.dma_start(out=wo_sb[:], in_=wo_d[:])
                for qc in range(4):
                    for c in range(4):
                        cs = slice(512 * qc, 512 * (qc + 1))
                        nc.gpsimd.dma_start(out=xt[c][:, cs],
                                            in_=xt_d[:, 2048 * c + 512 * qc:
                                                     2048 * c + 512 * (qc + 1)])

                qt = [persist.tile([128, 2048], f32r, tag=f"qt{m}", name=f"qt{m}")
                      for m in range(2)]
                kt = [persist.tile([128, 2048], f32r, tag=f"kt{m}", name=f"kt{m}")
                      for m in range(2)]
                v2 = [persist.tile([128, 512], bf16, tag=f"v{j}", name=f"v{j}")
                      for j in range(8)]
                ot = [persist.tile([128, 2048], f32r, tag=f"ot{m}", name=f"ot{m}")
                      for m in range(2)]

                # ---- projection tasks ----
                def proj_kq(w_sb, dst, p, qc, pool, tag):
                    ps = pool.tile([128, 512], f32, tag=tag, name="pp")
                    for vkc in range(4):
                        mm(ps[:, 0:512],
                           w_sb[:, 256 * vkc + 128 * p:256 * vkc + 128 * p + 128],
                           xt[vkc][:, 512 * qc:512 * (qc + 1)],
                           start=(vkc == 0), stop=(vkc == 3))
                    nc.vector.tensor_copy(dst[:, 512 * qc:512 * (qc + 1)],
                                          ps[:, 0:512])

                def proj_v(j, pool, tag):
                    # V for key chunks (2j, 2j+1): two sequential accumulation
                    # groups in one bank (second start=True only resets
                    # has_written bits; finished values in cols 0:256 persist)
                    ps = pool.tile([128, 512], f32, tag=tag, name="pv")
                    for half in range(2):
                        kc = 2 * j + half
                        for vkc in range(4):
                            mm(ps[:, 256 * half:256 * half + 256],
                               xt[vkc][:, 128 * kc:128 * (kc + 1)],
                               wv_sb[:, 256 * vkc:256 * (vkc + 1)],
                               start=(vkc == 0), stop=(vkc == 3))
                    nc.vector.tensor_copy(v2[j][:], ps[:, 0:512])

                # All projections up front in emission (the list scheduler
                # interleaves them with attention), rotating over the three
                # non-ring banks: pj + the two attnV banks (psO allocations
                # rotate in behind the projections on the same tags).
                def run_proj(t, pool, tag):
                    if t[0] == "k":
                        proj_kq(wk_sb, kt[t[1]], t[1], t[2], pool, tag)
                    elif t[0] == "q":
                        proj_kq(wq_sb, qt[t[1]], t[1], t[2], pool, tag)
                    else:
                        proj_v(t[1], pool, tag)

                # Lead projections: the ones the first scores block and the
                # first attnV units need. oA/oB are only used BEFORE the
                # first psO allocation on those tags (later use would wedge
                # the bufs=1 rotation behind a full-qc psO lifetime).
                run_proj(("k", 0, 0), ppsum, "pj")
                run_proj(("q", 0, 0), opsum, "oA")
                # V0-V4 stream on the oA/oB chains from inside the first
                # blocks (emitted by the attention loop below)
                ov_tasks = [("v", 0, opsum, "oB"), ("v", 1, opsum, "oA"),
                            ("v", 2, opsum, "oB"), ("v", 3, opsum, "oA"),
                            ("v", 4, opsum, "oB")]
                # Everything else drips down the pj bank, interleaved into
                # the attention emission (greedy scheduler: emission order is
                # priority, so pacing them keeps ACT fed).
                pj_tasks = iter([
                    ("k", 0, 1), ("k", 0, 2), ("k", 0, 3),
                    ("v", 5), ("v", 6), ("v", 7), ("q", 0, 1),
                    ("q", 0, 2), ("q", 0, 3),
                    ("k", 1, 0), ("k", 1, 1), ("k", 1, 2), ("k", 1, 3),
                    ("q", 1, 0), ("q", 1, 1), ("q", 1, 2), ("q", 1, 3)])
                v_emitted = set()

                # ---- output projection half-waves ([128 tokens, 512]) ----
                TAILROT = [(ppsum, "pj"), (rpsum, "rA"), (rpsum, "rB"),
                           (ppsum, "pj")]

                def p_half(t2, last=False):
                    pool, tag = TAILROT[t2 % 4] if last else (ppsum, "pj")
                    ps = pool.tile([128, 512], f32, tag=tag, name="pw")
                    for c in range(2):
                        mm(ps[:, 0:512], ot[c][:, 128 * t2:128 * (t2 + 1)],
                           wo_sb[:, 512 * c:512 * (c + 1)],
                           start=(c == 0), stop=(c == 1))
                    st = stage.tile([128, 512], f32, tag="pstage", name="pstage")
                    nc.vector.tensor_copy(st[:], ps[:, 0:512])
                    nc.sync.dma_start(out=out_d[128 * t2:128 * (t2 + 1), :],
                                      in_=st[:])

                # ---- attention ----
                SIZES = [3, 2] * 6 + [2]  # units of 512 cols; 3->rA, 2->rB

                # attnV machinery: a global FIFO of (p, qc, kc, h, attn-
                # tile, col) drained at most DRAIN units per block, so the
                # DMA/V-projection-induced backlog from early query chunks
                # spreads across later blocks instead of stalling ScalarE at
                # qc boundaries. psO alloc/evac happen lazily in-queue.
                DRAIN = 3
                avq = []
                po_state = {"cur": None, "psA": None, "psB": None}

                def emit_evac_and_pwaves():
                    pp, qq = po_state["cur"]
                    qs2 = slice(512 * qq, 512 * (qq + 1))
                    nc.vector.tensor_copy(ot[pp][0:64, qs2],
                                          po_state["psA"][0:64, 0:512])
                    nc.vector.tensor_copy(ot[pp][64:128, qs2],
                                          po_state["psB"][64:128, 0:512])
                    if pp == 1:
                        if qq == 3:
                            for t2 in range(12, 16):
                                p_half(t2, last=True)
                        else:
                            pw_pend.extend(range(4 * qq, 4 * qq + 4))

                def drain_avq(limit, keep=0):
                    n = 0
                    while len(avq) > keep:
                        if limit is not None and n >= limit:
                            break
                        pp, qq, kc, h, at, ci = avq[0]
                        if kc // 2 not in v_emitted:
                            break
                        if po_state["cur"] != (pp, qq):
                            if po_state["cur"] is not None:
                                emit_evac_and_pwaves()
                            po_state["cur"] = (pp, qq)
                            po_state["psA"] = opsum.tile([128, 512], f32,
                                                         tag="oA", name="psA")
                            po_state["psB"] = opsum.tile([128, 512], f32,
                                                         tag="oB", name="psB")
                        psX = po_state["psA"] if h == 0 else po_state["psB"]
                        vcol = 256 * (kc % 2) + 128 * p_glob(pp) + 64 * h
                        mmb(psX[64 * h:64 * h + 64, 0:512],
                            v2[kc // 2][:, vcol:vcol + 64],
                            at[:, 512 * ci:512 * (ci + 1)],
                            start=(kc == 0), stop=(kc == 15))
                        avq.pop(0)
                        n += 1

                def p_glob(pp):
                    return pp

                def attention(p):
                    for qc in range(4):
                        qs = slice(512 * qc, 512 * (qc + 1))
                        units = [(kc, h) for kc in range(16) for h in (0, 1)]
                        ui = 0
                        for bi, bs in enumerate(SIZES):
                            if bi % 2 == 0:
                                t = next(pj_tasks, None)
                                if t is not None:
                                    run_proj(t, ppsum, "pj")
                                    if t[0] == "v":
                                        v_emitted.add(t[1])
                            for _ in range(2):
                                if ov_tasks:
                                    ovt = ov_tasks.pop(0)
                                    run_proj(ovt[:2], ovt[2], ovt[3])
                                    v_emitted.add(ovt[1])
                            if pw_pend and bi % 2 == 1:
                                p_half(pw_pend.pop(0))
                            tag = "rA" if bs == 3 else "rB"
                            atag = "aA" if bs == 3 else "aB"
                            cur = units[ui:ui + bs]
                            ui += bs
                            s = rpsum.tile([128, 512 * bs], f32, tag=tag,
                                           name="ps")
                            for ci, (kc, h) in enumerate(cur):
                                mm(s[:, 512 * ci:512 * (ci + 1)],
                                   kt[p][64 * h:64 * h + 64,
                                         128 * kc:128 * (kc + 1)],
                                   qt[p][64 * h:64 * h + 64, qs],
                                   start=True, stop=True)
                            a = apool.tile([128, 512 * bs], bf16, tag=atag,
                                           name="attn")
                            nc.scalar.activation(a[:], s[:], Sig,
                                                 bias=bias_t[:],
                                                 scale=INV_SQRT_DK)
                            for ci, (kc, h) in enumerate(cur):
                                avq.append((p, qc, kc, h, a, ci))
                            if not ov_tasks:
                                # (first psO alloc must come after every
                                # oA/oB-tagged projection's emission)
                                keep = 0 if (p, qc) == (1, 3) else 4
                                drain_avq(DRAIN, keep=keep)

                pw_pend = []
                attention(0)
                attention(1)
                drain_avq(None)
                emit_evac_and_pwaves()
                while pw_pend:
                    p_half(pw_pend.pop(0))

                if DEBUG:
                    for m in range(2):
                        nc.sync.dma_start(out=dbg["qt"][m], in_=qt[m][:])
                        nc.sync.dma_start(out=dbg["kt"][m], in_=kt[m][:])
                        nc.sync.dma_start(out=dbg["ot"][m], in_=ot[m][:])
                    for j in range(8):
                        nc.sync.dma_start(out=dbg["v"][j], in_=v2[j][:])

    nc.compile()
    return nc


def get_nc():
    if "nc" not in _CACHE:
        _CACHE["nc"] = _build_nc()
    return _CACHE["nc"]


def make_in_maps(x, W_q, W_k, W_v, W_o):
    x = np.ascontiguousarray(np.asarray(x, dtype=np.float32))
    ws = [np.asarray(w, dtype=np.float32) for w in (W_q, W_k, W_v, W_o)]
    W_q, W_k, W_v, W_o = ws

    def chunked(a, nchunks):
        # [128*nchunks, m] -> [128, nchunks*m] with chunk-major columns
        m = a.shape[1]
        return np.ascontiguousarray(
            a.reshape(nchunks, 128, m).transpose(1, 0, 2).reshape(128, nchunks * m))

    in_maps = []
    for c in range(8):
        b, g = divmod(c, 2)
        gf = slice(GF * g, GF * (g + 1))
        in_maps.append({
            "xt": chunked(np.ascontiguousarray(x[b].T), 4),
            "wq": chunked(np.ascontiguousarray(W_q[gf, :].T), 4),
            "wk": chunked(np.ascontiguousarray(W_k[gf, :].T), 4),
            "wv": chunked(np.ascontiguousarray(W_v[gf, :].T), 4),
            "wo": chunked(np.ascontiguousarray(W_o[:, gf].T), 2),
        })
    return in_maps


def kernel(x, W_q, W_k, W_v, W_o):
    from concourse.bass_utils import run_bass_kernel_spmd

    nc = get_nc()
    in_maps = make_in_maps(x, W_q, W_k, W_v, W_o)
    res = run_bass_kernel_spmd(nc, in_maps, list(range(8)))
    parts = [res.results[c]["out"] for c in range(8)]
    out = np.stack([parts[2 * b] + parts[2 * b + 1] for b in range(B)])
    return np.ascontiguousarray(out.astype(np.float32))


# revision 12
# speedup vs baseline: 1.3455x; 1.0281x over previous
"""Sigmoid-attention MHA kernel for 8 Trainium2 NeuronCores (v7).

Problem: x[4,2048,512], W_q/W_k/W_v/W_o[512,512] (already scaled).
  Q = x@Wq.T, K = x@Wk.T, V = x@Wv.T split into 8 heads of depth 64
  attn = sigmoid(QK^T/sqrt(64) - log(2048));  out = (attn@V merged)@Wo.T

Sharding: core c handles batch b=c//2, head-group g=c%2 (4 heads = 2 pairs).
Each core computes a partial output projection over its 256 head-features;
host sums the two partials per batch.

v2 design (ScalarE sigmoid is the bottleneck: ~16.8M activations/core at
1 elem/cycle/lane @1.2GHz ~= 110us + per-instruction overhead):
  - PSUM (8 banks): score ring rA[128,1536](3) + rB[128,1024](2), one
    projection bank pj[128,512], two col-tiled attnV banks oA/oB.
  - Bigger ACTIVATE blocks (1536/1024 alternating) cut per-call overhead.
  - Scores row-tiled (two 64-contraction heads in PE row halves), attnV
    col-tiled (head outputs at PSUM partitions 0-63 / 64-127) -> PE ~2x
    on attention matmuls, keeping PE well under ScalarE.
  - x DMA'd in query-chunk order; only K/Q(pair0,qc0) block the first
    scores; all other projections stream on the pj bank concurrently
    with attention.
  - Output projection runs as [128,512] half-waves on the pj bank as soon
    as each pair-1 query chunk retires; the last 4 rotate over free banks.
  - attn/V in bf16 (fp32 weight load fails the ISA check under column
    tiling; fp32 PSUM accumulation keeps rounding benign).
"""

import os
import numpy as np

DEBUG = bool(int(os.environ.get("KERNEL_DEBUG", "0")))
LOOP = int(os.environ.get("KERNEL_LOOP", "0"))  # >0: wrap body in For_i (timing)
ABUFS = int(os.environ.get("KERNEL_ABUFS", "6"))

B, S, D = 4, 2048, 512
NH, DEPTH = 8, 64
G = 2          # head groups (one per core pair)
GF = 256       # features per group
NEG_LOG_S = float(np.float32(-np.log(np.float32(S))))
INV_SQRT_DK = 0.125

_CACHE = {}


def _build_nc():
    import concourse.bacc as bacc
    import concourse.tile as tile
    from concourse import mybir

    f32 = mybir.dt.float32
    f32r = mybir.dt.float32r
    bf16 = mybir.dt.bfloat16
    nc = bacc.Bacc("TRN2", target_bir_lowering=False, debug=False, num_devices=8)

    xt_d = nc.dram_tensor("xt", [128, 8192], f32r, kind="ExternalInput").ap()
    wq_d = nc.dram_tensor("wq", [128, 1024], f32r, kind="ExternalInput").ap()
    wk_d = nc.dram_tensor("wk", [128, 1024], f32r, kind="ExternalInput").ap()
    wv_d = nc.dram_tensor("wv", [128, 1024], f32r, kind="ExternalInput").ap()
    wo_d = nc.dram_tensor("wo", [128, 1024], f32r, kind="ExternalInput").ap()
    out_d = nc.dram_tensor("out", [S, D], f32, kind="ExternalOutput").ap()
    dbg = {}
    if DEBUG:
        for nm in ("qt", "kt"):
            dbg[nm] = [nc.dram_tensor(f"dbg_{nm}{m}", [128, 2048], f32r,
                                      kind="ExternalOutput").ap() for m in range(2)]
        dbg["ot"] = [nc.dram_tensor(f"dbg_ot{m}", [128, 2048], f32r,
                                    kind="ExternalOutput").ap() for m in range(2)]
        dbg["v"] = [nc.dram_tensor(f"dbg_v{j}", [128, 512], bf16,
                                   kind="ExternalOutput").ap() for j in range(8)]

    with tile.TileContext(nc) as tc:
        with (
            tc.tile_pool(name="persist", bufs=1) as persist,
            tc.tile_pool(name="attn", bufs=ABUFS) as apool,
            tc.tile_pool(name="stage", bufs=3) as stage,
            tc.tile_pool(name="rpsum", bufs=1, space="PSUM") as rpsum,
            tc.tile_pool(name="ppsum", bufs=1, space="PSUM") as ppsum,
            tc.tile_pool(name="opsum", bufs=1, space="PSUM") as opsum,
        ):
            import contextlib
            if LOOP > 0:
                loop_cm = tc.For_i(0, LOOP, 1)
            else:
                loop_cm = contextlib.nullcontext()
            Sig = mybir.ActivationFunctionType.Sigmoid

            def mm(out, lhsT, rhs, start, stop):
                # float32r: single-pass fp32 matmul, slightly reduced
                # multiply precision
                nc.tensor.matmul(out, lhsT=lhsT.bitcast(f32r),
                                 rhs=rhs.bitcast(f32r), start=start, stop=stop)

            def mmb(out, lhsT, rhs, start, stop):
                nc.tensor.matmul(out, lhsT=lhsT, rhs=rhs,
                                 start=start, stop=stop)

            with loop_cm:
                bias_t = persist.tile([128, 1], f32, tag="bias", name="bias_t")
                nc.vector.memset(bias_t[:], NEG_LOG_S)
                warm_t = persist.tile([128, 1], f32, tag="warm", name="warm_t")
                nc.scalar.activation(warm_t[:], bias_t[:], Sig, bias=bias_t[:])

                wq_sb = persist.tile([128, 1024], f32r, tag="wq", name="wq_sb")
                wk_sb = persist.tile([128, 1024], f32r, tag="wk", name="wk_sb")
                wv_sb = persist.tile([128, 1024], f32r, tag="wv", name="wv_sb")
                wo_sb = persist.tile([128, 1024], f32r, tag="wo", name="wo_sb")
                xt = [persist.tile([128, 2048], f32r, tag=f"xt{c}", name=f"xt{c}")
                      for c in range(4)]
                # Weights for K/Q first, then x in query-chunk-major order so
                # the pair-0 qc0 projections can start after ~1MiB of x.
                # Few, large DMAs (per-DMA overhead on HW is much larger
                # than the bandwidth model suggests): weights first, then x
                # in two half-token sweeps so the first projections can
                # start after ~half the x traffic.
                nc.sync.dma_start(out=wk_sb[:], in_=wk_d[:])
                nc.sync.dma_start(out=wq_sb[:], in_=wq_d[:])
                for c in range(4):
                    nc.sync.dma_start(out=xt[c][:, 0:1024],
                                      in_=xt_d[:, 2048 * c:2048 * c + 1024])
                nc.sync.dma_start(out=wv_sb[:], in_=wv_d[:])
                for c in range(4):
                    nc.sync.dma_start(out=xt[c][:, 1024:2048],
                                      in_=xt_d[:, 2048 * c + 1024:2048 * (c + 1)])
                nc.sync.dma_start(out=wo_sb[:], in_=wo_d[:])

                qt = [persist.tile([128, 2048], f32r, tag=f"qt{m}", name=f"qt{m}")
                      for m in range(2)]
                kt = [persist.tile([128, 2048], f32r, tag=f"kt{m}", name=f"kt{m}")
                      for m in range(2)]
                v2 = [persist.tile([128, 512], bf16, tag=f"v{j}", name=f"v{j}")
                      for j in range(8)]
                ot = [persist.tile([128, 2048], f32r, tag=f"ot{m}", name=f"ot{m}")
                      for m in range(2)]

                # ---- projection tasks ----
                def proj_kq(w_sb, dst, p, qc, pool, tag):
                    ps = pool.tile([128, 512], f32, tag=tag, name="pp")
                    for vkc in range(4):
                        mm(ps[:, 0:512],
                           w_sb[:, 256 * vkc + 128 * p:256 * vkc + 128 * p + 128],
                           xt[vkc][:, 512 * qc:512 * (qc + 1)],
                           start=(vkc == 0), stop=(vkc == 3))
                    nc.vector.tensor_copy(dst[:, 512 * qc:512 * (qc + 1)],
                                          ps[:, 0:512])

                def proj_v(j, pool, tag):
                    # V for key chunks (2j, 2j+1): two sequential accumulation
                    # groups in one bank (second start=True only resets
                    # has_written bits; finished values in cols 0:256 persist)
                    ps = pool.tile([128, 512], f32, tag=tag, name="pv")
                    for half in range(2):
                        kc = 2 * j + half
                        for vkc in range(4):
                            mm(ps[:, 256 * half:256 * half + 256],
                               xt[vkc][:, 128 * kc:128 * (kc + 1)],
                               wv_sb[:, 256 * vkc:256 * (vkc + 1)],
                               start=(vkc == 0), stop=(vkc == 3))
                    nc.vector.tensor_copy(v2[j][:], ps[:, 0:512])

                # All projections up front in emission (the list scheduler
                # interleaves them with attention), rotating over the three
                # non-ring banks: pj + the two attnV banks (psO allocations
                # rotate in behind the projections on the same tags).
                def run_proj(t, pool, tag):
                    if t[0] == "k":
                        proj_kq(wk_sb, kt[t[1]], t[1], t[2], pool, tag)
                    elif t[0] == "q":
                        proj_kq(wq_sb, qt[t[1]], t[1], t[2], pool, tag)
                    else:
                        proj_v(t[1], pool, tag)

                # Lead projections: the ones the first scores block and the
                # first attnV units need. oA/oB are only used BEFORE the
                # first psO allocation on those tags (later use would wedge
                # the bufs=1 rotation behind a full-qc psO lifetime).
                run_proj(("k", 0, 0), ppsum, "pj")
                run_proj(("q", 0, 0), opsum, "oA")
                # V0-V4 stream on the oA/oB chains from inside the first
                # blocks (emitted by the attention loop below)
                ov_tasks = [("v", 0, opsum, "oB"), ("v", 1, opsum, "oA"),
                            ("v", 2, opsum, "oB"), ("v", 3, opsum, "oA"),
                            ("v", 4, opsum, "oB")]
                # Everything else drips down the pj bank, interleaved into
                # the attention emission (greedy scheduler: emission order is
                # priority, so pacing them keeps ACT fed).
                pj_tasks = iter([
                    ("k", 0, 1), ("k", 0, 2), ("k", 0, 3),
                    ("v", 5), ("v", 6), ("v", 7), ("q", 0, 1),
                    ("q", 0, 2), ("q", 0, 3),
                    ("k", 1, 0), ("k", 1, 1), ("k", 1, 2), ("k", 1, 3),
                    ("q", 1, 0), ("q", 1, 1), ("q", 1, 2), ("q", 1, 3)])
                v_emitted = set()

                # ---- output projection half-waves ([128 tokens, 512]) ----
                TAILROT = [(ppsum, "pj"), (rpsum, "rA"), (rpsum, "rB"),
                           (ppsum, "pj")]

                pw_stage = {}

                def p_half(t2, last=False):
                    pool, tag = TAILROT[t2 % 4] if last else (ppsum, "pj")
                    ps = pool.tile([128, 512], f32, tag=tag, name="pw")
                    for c in range(2):
                        mm(ps[:, 0:512], ot[c][:, 128 * t2:128 * (t2 + 1)],
                           wo_sb[:, 512 * c:512 * (c + 1)],
                           start=(c == 0), stop=(c == 1))
                    w, half = divmod(t2, 2)
                    if half == 0:
                        pw_stage[w] = stage.tile([128, 2, 512], f32,
                                                 tag="pstage", name="pstage")
                    st = pw_stage[w]
                    nc.vector.tensor_copy(st[:, half, :], ps[:, 0:512])
                    if half == 1:
                        dst = out_d[256 * w:256 * (w + 1), :].rearrange(
                            "(t p) m -> p t m", p=128)
                        nc.sync.dma_start(out=dst, in_=pw_stage.pop(w)[:])

                # ---- attention ----
                SIZES = [3, 2] * 6 + [2]  # units of 512 cols; 3->rA, 2->rB

                # attnV machinery: a global FIFO of (p, qc, kc, h, attn-
                # tile, col) drained at most DRAIN units per block, so the
                # DMA/V-projection-induced backlog from early query chunks
                # spreads across later blocks instead of stalling ScalarE at
                # qc boundaries. psO alloc/evac happen lazily in-queue.
                DRAIN = 3
                avq = []
                po_state = {"cur": None, "psA": None, "psB": None}

                def emit_evac_and_pwaves():
                    pp, qq = po_state["cur"]
                    qs2 = slice(512 * qq, 512 * (qq + 1))
                    nc.vector.tensor_copy(ot[pp][0:64, qs2],
                                          po_state["psA"][0:64, 0:512])
                    nc.vector.tensor_copy(ot[pp][64:128, qs2],
                                          po_state["psB"][64:128, 0:512])
                    if pp == 1:
                        if qq == 3:
                            for t2 in range(12, 16):
                                p_half(t2, last=True)
                        else:
                            pw_pend.extend(range(4 * qq, 4 * qq + 4))

                def drain_avq(limit, keep=0):
                    n = 0
                    while len(avq) > keep:
                        if limit is not None and n >= limit:
                            break
                        pp, qq, kc, h, at, ci = avq[0]
                        if kc // 2 not in v_emitted:
                            break
                        if po_state["cur"] != (pp, qq):
                            if po_state["cur"] is not None:
                                emit_evac_and_pwaves()
                            po_state["cur"] = (pp, qq)
                            po_state["psA"] = opsum.tile([128, 512], f32,
                                                         tag="oA", name="psA")
                            po_state["psB"] = opsum.tile([128, 512], f32,
                                                         tag="oB", name="psB")
                        psX = po_state["psA"] if h == 0 else po_state["psB"]
                        vcol = 256 * (kc % 2) + 128 * p_glob(pp) + 64 * h
                        mmb(psX[64 * h:64 * h + 64, 0:512],
                            v2[kc // 2][:, vcol:vcol + 64],
                            at[:, 512 * ci:512 * (ci + 1)],
                            start=(kc == 0), stop=(kc == 15))
                        avq.pop(0)
                        n += 1

                def p_glob(pp):
                    return pp

                def attention(p):
                    for qc in range(4):
                        qs = slice(512 * qc, 512 * (qc + 1))
                        units = [(kc, h) for kc in range(16) for h in (0, 1)]
                        ui = 0
                        for bi, bs in enumerate(SIZES):
                            if bi % 2 == 0:
                                t = next(pj_tasks, None)
                                if t is not None:
                                    run_proj(t, ppsum, "pj")
                                    if t[0] == "v":
                                        v_emitted.add(t[1])
                            for _ in range(2):
                                if ov_tasks:
                                    ovt = ov_tasks.pop(0)
                                    run_proj(ovt[:2], ovt[2], ovt[3])
                                    v_emitted.add(ovt[1])
                            if pw_pend and bi % 2 == 1:
                                p_half(pw_pend.pop(0))
                            tag = "rA" if bs == 3 else "rB"
                            atag = "aA" if bs == 3 else "aB"
                            cur = units[ui:ui + bs]
                            ui += bs
                            s = rpsum.tile([128, 512 * bs], f32, tag=tag,
                                           name="ps")
                            for ci, (kc, h) in enumerate(cur):
                                mm(s[:, 512 * ci:512 * (ci + 1)],
                                   kt[p][64 * h:64 * h + 64,
                                         128 * kc:128 * (kc + 1)],
                                   qt[p][64 * h:64 * h + 64, qs],
                                   start=True, stop=True)
                            a = apool.tile([128, 512 * bs], bf16, tag=atag,
                                           name="attn")
                            nc.scalar.activation(a[:], s[:], Sig,
                                                 bias=bias_t[:],
                                                 scale=INV_SQRT_DK)
                            for ci, (kc, h) in enumerate(cur):
                                avq.append((p, qc, kc, h, a, ci))
                            if not ov_tasks:
                                # (first psO alloc must come after every
                                # oA/oB-tagged projection's emission)
                                keep = 0 if (p, qc) == (1, 3) else 4
                                drain_avq(DRAIN, keep=keep)

                pw_pend = []
                attention(0)
                attention(1)
                drain_avq(None)
                emit_evac_and_pwaves()
                while pw_pend:
                    p_half(pw_pend.pop(0))

                if DEBUG:
                    for m in range(2):
                        nc.sync.dma_start(out=dbg["qt"][m], in_=qt[m][:])
                        nc.sync.dma_start(out=dbg["kt"][m], in_=kt[m][:])
                        nc.sync.dma_start(out=dbg["ot"][m], in_=ot[m][:])
                    for j in range(8):
                        nc.sync.dma_start(out=dbg["v"][j], in_=v2[j][:])

    nc.compile()
    return nc


def get_nc():
    if "nc" not in _CACHE:
        _CACHE["nc"] = _build_nc()
    return _CACHE["nc"]


def make_in_maps(x, W_q, W_k, W_v, W_o):
    x = np.ascontiguousarray(np.asarray(x, dtype=np.float32))
    ws = [np.asarray(w, dtype=np.float32) for w in (W_q, W_k, W_v, W_o)]
    W_q, W_k, W_v, W_o = ws

    def chunked(a, nchunks):
        # [128*nchunks, m] -> [128, nchunks*m] with chunk-major columns
        m = a.shape[1]
        return np.ascontiguousarray(
            a.reshape(nchunks, 128, m).transpose(1, 0, 2).reshape(128, nchunks * m))

    in_maps = []
    for c in range(8):
        b, g = divmod(c, 2)
        gf = slice(GF * g, GF * (g + 1))
        in_maps.append({
            "xt": chunked(np.ascontiguousarray(x[b].T), 4),
            "wq": chunked(np.ascontiguousarray(W_q[gf, :].T), 4),
            "wk": chunked(np.ascontiguousarray(W_k[gf, :].T), 4),
            "wv": chunked(np.ascontiguousarray(W_v[gf, :].T), 4),
            "wo": chunked(np.ascontiguousarray(W_o[:, gf].T), 2),
        })
    return in_maps


def kernel(x, W_q, W_k, W_v, W_o):
    from concourse.bass_utils import run_bass_kernel_spmd

    nc = get_nc()
    in_maps = make_in_maps(x, W_q, W_k, W_v, W_o)
    res = run_bass_kernel_spmd(nc, in_maps, list(range(8)))
    parts = [res.results[c]["out"] for c in range(8)]
    out = np.stack([parts[2 * b] + parts[2 * b + 1] for b in range(B)])
    return np.ascontiguousarray(out.astype(np.float32))
